# revision 9
# baseline (speedup 1.0000x reference)
"""Trainium2 Bass kernel for nn_AvgPoolVectorsPerWSI (segment-mean over groups).

Math: x [N=2048, M=512, 7, 7], idx [N] in [0,64)
  out[g, m] = mean over {n: idx[n]==g} and spatial of x[n, m, :, :]  -> [64, 512, 1, 1]

Strategy (no collectives needed):
  - Shard over M: core k handles an m-slice of 64 channels. Each core reads
    its x slice [2048, 64, 49] (25.7 MB) once; the stream runs at the SDMA
    port limit (~26 GB/s/engine, 12.5 KB packets) -> ~62-65 us.
  - SDMA engine 15 intermittently runs ~20-45% slow (known HW issue) and
    would gate every transfer's completion. Engine k serves fixed SBUF
    partitions; engine 15 owns {92-95, 124-127}. So rows are packed into 17
    tiles: 12 FULL (128 rows, all partitions), 4 LITE (120 rows, partitions
    [0:92)+[96:124) -> engine 15 idle), 1 MINI (32 rows). Engine 15 carries
    96 of 2048 rows and the last two tiles never touch it, so a slow engine
    15 stops mattering. Unused partitions get w == 0 (idx = -1 in aux), so
    their stale SBUF content contributes nothing.
  - All compute is fp32-exact. Per tile:
      * VectorE, m-channels [0, MV): spatial j-reduce to xs[n, m] (2 halves),
        then TensorE accumulates psum_small[g, m] += w[n, g]^T @ xs[n, m].
      * TensorE, m-channels [MV, 64): fused segment-sum directly on raw x,
        psum_big[g, (m,j)] += w[n, g]^T @ x[n, (m,j)] in 4 psum chunks.
    with w the scale-weighted one-hot (scale = 1/(count_g*49)), generated ON
    DEVICE from a 74 KB aux tensor so the HBM stream is just x.
  - Tail pipelining: the last two tiles are DMA'd in pieces (vec halves
    first, PE chunks last; one semaphore PER PIECE — cumulative counts race
    across in-flight pieces because the 16 SDMA engines' +1s interleave).
    ScalarE copies psum_small -> out_sb (PSUM port, otherwise idle; its ACT
    Copy table is pre-warmed — the first activation pays a ~1.3us table
    load). VectorE j-reduces psum_big as the last matmul chunks complete.
    aux rides ScalarE's separate HWDGE queue so the x stream starts on the
    sync engine's first instruction.
  - PSUM rules honored: one accumulation start per BANK (start=True clears
    whole-bank accumulate bits); VectorE only reads a bank once no more
    matmuls will write it (same-bank PE-write + DVE-read is fatal).

Raw Block implementation (not Tile): the walrus matmul/DMA lowerings only
accept ONE attached sync-wait per instruction; standalone wait_ge
instructions sidestep that.
"""

from contextlib import ExitStack

import numpy as np

import concourse.bass as bass
import concourse.mybir as mybir
from concourse.bass_utils import run_bass_kernel_spmd

N = 2048          # samples
M = 512           # channels
HW = 49           # spatial (7*7)
G = 64            # groups
CORES = 8
ML = M // CORES   # 64 channels per core
F = ML * HW       # 3136 floats per (n, core)
P = 128           # partitions
BUFS = 8          # x-tile buffer depth

MV = 44           # m-channels through VectorE spatial-reduce + small matmul
MP = ML - MV      # 20 m-channels through TensorE raw fused matmul
FV = MV * HW      # 2156 vec-path columns
FP = MP * HW      # 980 raw columns through the PE
VH = (MV // 2) * HW  # 1078: half of the vec region (reduce granularity)

# --- tile schedule: de-load SDMA engine 15 (partitions 92-95, 124-127) ---
# partition ranges (dest) per tile kind; rows pack consecutively into ranges
FULL = ((0, 128),)
LITE = ((0, 92), (96, 124))
MINI = ((36, 64), (96, 100))
TILE_KINDS = [MINI] + [FULL] * 3 + [LITE] + [FULL] * 3 + [LITE] + [FULL] * 6 \
    + [LITE, LITE]
NTILES = len(TILE_KINDS)  # 17
TILE_ROWS = [sum(hi - lo for lo, hi in k) for k in TILE_KINDS]
assert sum(TILE_ROWS) == N and NTILES == 17
TILE_START = np.concatenate([[0], np.cumsum(TILE_ROWS)[:-1]]).tolist()

# fp32 matmul chunks within psum banks (bank 0 = cols [0:512), bank 1 =
# [512:980)). Only the first chunk per bank sets start=True at t=0.
CHUNKS = [(0, 256), (256, 512), (512, 768), (768, FP)]
NCH = len(CHUNKS)
CHUNK_START = {0, 2}

# epilogue j-reduce of psum_big: (0,10) reads bank 0 only -> after the last
# tile's chunk 1; (10,20) spans both banks -> after ALL matmuls.
SUBRED = [
    (0, 10, (NTILES - 1) * NCH + 2),
    (10, MP, NTILES * NCH),
]

# DMA pieces (column ranges) per tile; last two tiles split for the tail.
PIECES = {t: [(0, F)] for t in range(NTILES)}
PIECES[NTILES - 2] = [(0, VH), (VH, FV), (FV, F)]
PIECES[NTILES - 1] = [(0, VH), (VH, FV)] + \
    [(FV + lo, FV + hi) for lo, hi in CHUNKS]

F32 = mybir.dt.float32


def _build(wait_out=True):
    nc = bass.Bass(trn_type="TRN2", target_bir_lowering=False)
    x_ext = nc.declare_dram_parameter("x", [N, F], F32, isOutput=False)
    # aux[:, 0:64] iota row, aux[:, 64:128] scale row, aux[:, 128:] per-tile
    # local row index (or -1 for unused partitions)
    aux_ext = nc.declare_dram_parameter("aux", [P, G + G + NTILES], F32,
                                        isOutput=False)
    out_ext = nc.declare_dram_parameter("out", [G, ML], F32, isOutput=True)

    xr = x_ext.ap()  # [N, F]

    with ExitStack() as ctx:
        x_buf = ctx.enter_context(nc.sbuf_tensor([P, BUFS * F], F32))
        xs_buf = ctx.enter_context(nc.sbuf_tensor([P, BUFS * MV], F32))
        aux_sb = ctx.enter_context(nc.sbuf_tensor([P, G + G + NTILES], F32))
        warm_sb = ctx.enter_context(nc.sbuf_tensor([G, 2], F32))
        w_sb = ctx.enter_context(nc.sbuf_tensor([P, NTILES * G], F32))
        out_sb = ctx.enter_context(nc.sbuf_tensor([G, ML], F32))
        psum_big = ctx.enter_context(nc.psum_tensor([G, FP], F32))
        psum_small = ctx.enter_context(nc.psum_tensor([G, MV], F32))
        # one sem per (tile, piece): piece k of tile t complete at
        # 16 * n_partition_ranges
        dma_x = {
            t: [
                ctx.enter_context(nc.semaphore(name=f"dx{t}_{k}"))
                for k in range(len(PIECES[t]))
            ]
            for t in range(NTILES)
        }
        dma_a = ctx.enter_context(nc.semaphore())   # +16 when aux resident
        dma_o = ctx.enter_context(nc.semaphore())   # +16 when out written
        wg_sem = ctx.enter_context(nc.semaphore())  # +1 when w generated
        red_sem = ctx.enter_context(nc.semaphore())  # +2 per tile j-reduce
        pe_big = ctx.enter_context(nc.semaphore())   # +1 per big matmul chunk
        pe_tile = ctx.enter_context(nc.semaphore())  # +1 per tile (small mm)
        fin_sem = ctx.enter_context(nc.semaphore())  # +3 when out_sb ready
        block = ctx.enter_context(nc.Block())

        def piece_done(engine, t, k):
            engine.wait_ge(dma_x[t][k], 16 * len(TILE_KINDS[t]))

        def vec_wait(engine, t, half):
            piece_done(engine, t, half if len(PIECES[t]) > 1 else 0)

        def pe_wait(engine, t, chunk):
            if len(PIECES[t]) == 1:
                k = 0
            elif t == NTILES - 2:
                k = 2
            else:
                k = 2 + chunk
            piece_done(engine, t, k)

        # ---- DMA program for x + out (SP / HWDGE, FIFO) ----
        @block.sync
        def _(sync):
            for t in range(NTILES):
                if t >= BUFS:
                    # slot reuse: small matmul is ordered after the tile's
                    # j-reduces and big matmuls
                    sync.wait_ge(pe_tile, t - BUFS + 1)
                slot = t % BUFS
                row = TILE_START[t]
                for k, (lo, hi) in enumerate(PIECES[t]):
                    for plo, phi in TILE_KINDS[t]:
                        nrows = phi - plo
                        sync.dma_start(
                            out=x_buf[plo:phi, slot * F + lo:slot * F + hi],
                            in_=xr[row:row + nrows, lo:hi],
                        ).then_inc(dma_x[t][k], 16)
                        row += nrows
                    row -= TILE_ROWS[t]
            sync.wait_ge(fin_sem, 3)
            sync.dma_start(out=out_ext.ap(), in_=out_sb[:, :]).then_inc(dma_o, 16)
            if wait_out:
                sync.wait_ge(dma_o, 16)

        # ---- ScalarE: aux DMA on the second HWDGE queue; psum_small copy ----
        @block.scalar
        def _(scalar):
            scalar.dma_start(out=aux_sb[:, :], in_=aux_ext.ap()).then_inc(dma_a, 16)
            # warm the ACT Copy PWP table now — the first activation pays a
            # ~1.3us ACT_TABLE_LOAD which must not hit the final-copy path
            scalar.copy(warm_sb[:, 0:1], warm_sb[:, 1:2])
            scalar.wait_ge(pe_tile, NTILES)
            scalar.copy(out_sb[:, 0:MV], psum_small[:, :]).then_inc(fin_sem, 1)

        # ---- VectorE: w generation, spatial j-reduction, psum_big epilogue ----
        @block.vector
        def _(vector):
            # scale-weighted one-hot from the per-tile local row index:
            #   w[p, t*G+g] = (idx[tile t, partition p] == g) * scale[g]
            vector.wait_ge(dma_a, 16)
            for t in range(NTILES):
                wg = vector.scalar_tensor_tensor(
                    out=w_sb[:, t * G:(t + 1) * G],
                    in0=aux_sb[:, 0:G],
                    scalar=aux_sb[:, 2 * G + t:2 * G + t + 1],
                    in1=aux_sb[:, G:2 * G],
                    op0=mybir.AluOpType.is_equal,
                    op1=mybir.AluOpType.mult,
                )
            wg.then_inc(wg_sem, 1)

            for t in range(NTILES):
                slot = t % BUFS
                if t >= BUFS:
                    # xs slot reuse: wait until tile t-BUFS consumed by PE
                    vector.wait_ge(pe_tile, t - BUFS + 1)
                for half in range(2):
                    vec_wait(vector, t, half)
                    vector.tensor_reduce(
                        out=xs_buf[:, slot * MV + half * (MV // 2):
                                   slot * MV + (half + 1) * (MV // 2)],
                        in_=x_buf[:, slot * F + half * VH:
                                  slot * F + (half + 1) * VH].rearrange(
                            "p (m j) -> p m j", j=HW
                        ),
                        axis=mybir.AxisListType.X,
                        op=mybir.AluOpType.add,
                    ).then_inc(red_sem, 1)

            # epilogue: j-reduce psum_big as the last tile's chunks complete
            for mlo, mhi, need in SUBRED:
                vector.wait_ge(pe_big, need)
                vector.tensor_reduce(
                    out=out_sb[:, MV + mlo:MV + mhi],
                    in_=psum_big[:, mlo * HW:mhi * HW].rearrange(
                        "p (m j) -> p m j", j=HW
                    ),
                    axis=mybir.AxisListType.X,
                    op=mybir.AluOpType.add,
                ).then_inc(fin_sem, 1)

        # ---- TensorE: segment-sum accumulation (fp32) ----
        @block.tensor
        def _(tensor):
            tensor.wait_ge(wg_sem, 1)
            for t in range(NTILES):
                slot = t % BUFS
                wt = w_sb[:, t * G:(t + 1) * G]
                prev_key = None
                for c, (lo, hi) in enumerate(CHUNKS):
                    key = (t, 2 + c) if t == NTILES - 1 else (t, 0)
                    if key != prev_key:
                        pe_wait(tensor, t, c)
                        prev_key = key
                    tensor.matmul(
                        out=psum_big[:, lo:hi],
                        lhsT=wt,
                        rhs=x_buf[:, slot * F + FV + lo:slot * F + FV + hi],
                        start=(t == 0 and c in CHUNK_START),
                        stop=(t == NTILES - 1),
                        skip_group_check=True,
                    ).then_inc(pe_big, 1)
                tensor.wait_ge(red_sem, 2 * (t + 1))
                tensor.matmul(
                    out=psum_small[:, :],
                    lhsT=wt,
                    rhs=xs_buf[:, slot * MV:(slot + 1) * MV],
                    start=(t == 0),
                    stop=(t == NTILES - 1),
                ).then_inc(pe_tile, 1)

    return nc


def _prepare(x, idx):
    x = np.asarray(x)
    if x.dtype != np.float32:
        x = x.astype(np.float32)
    idx = np.asarray(idx).astype(np.int64)
    counts = np.bincount(idx, minlength=G).astype(np.float64)
    scale = np.where(counts > 0, 1.0 / (counts * HW), 0.0).astype(np.float32)
    aux = np.zeros((P, G + G + NTILES), np.float32)
    aux[:, 0:G] = np.arange(G, dtype=np.float32)[None, :]
    aux[:, G:2 * G] = scale[None, :]
    # per-tile: partition p holds row TILE_START[t] + local(p), or no row
    # (idx -1 -> w row of zeros kills the stale SBUF contents)
    for t in range(NTILES):
        col = np.full(P, -1.0, np.float32)
        row = TILE_START[t]
        for plo, phi in TILE_KINDS[t]:
            nrows = phi - plo
            col[plo:phi] = idx[row:row + nrows].astype(np.float32)
            row += nrows
        aux[:, 2 * G + t] = col
    xr = x.reshape(N, M, HW)
    in_maps = []
    for k in range(CORES):
        shard = np.ascontiguousarray(xr[:, k * ML:(k + 1) * ML, :]).reshape(N, F)
        in_maps.append({"x": shard, "aux": aux})
    return in_maps


def run(x, tensor_list_assignmentindices, trace=False, wait_out=True):
    in_maps = _prepare(x, tensor_list_assignmentindices)
    nc = _build(wait_out=wait_out)
    res = run_bass_kernel_spmd(nc, in_maps, core_ids=list(range(CORES)), trace=trace)
    outs = [np.asarray(r["out"]) for r in res.results]
    out = np.concatenate(outs, axis=1)  # [G, M]
    return out.reshape(G, M, 1, 1).astype(np.float32), res.exec_time_ns


def kernel(**inputs):
    out, _ = run(inputs["x"], inputs["tensor_list_assignmentindices"], trace=False)
    return out


# revision 10
# speedup vs baseline: 1.5240x; 1.5240x over previous
"""Trainium2 Bass kernel for nn_AvgPoolVectorsPerWSI (segment-mean over groups).

Math: x [N=2048, M=512, 7, 7], idx [N] in [0,64)
  out[g, m] = mean over {n: idx[n]==g} and spatial of x[n, m, :, :]  -> [64, 512, 1, 1]

Strategy (no collectives needed):
  - Shard over M: core k handles an m-slice of 64 channels. Each core reads
    its x slice [2048, 64, 49] (25.7 MB) once; the stream runs at the SDMA
    port limit (~26 GB/s/engine, 12.5 KB packets) -> ~62-65 us.
  - SDMA engine 15 intermittently runs ~20-45% slow (known HW issue) and
    would gate every transfer's completion. Engine k serves fixed SBUF
    partitions; engine 15 owns {92-95, 124-127}. So rows are packed into 17
    tiles: 12 FULL (128 rows, all partitions), 4 LITE (120 rows, partitions
    [0:92)+[96:124) -> engine 15 idle), 1 MINI (32 rows). Engine 15 carries
    96 of 2048 rows and the last two tiles never touch it, so a slow engine
    15 stops mattering. Unused partitions get w == 0 (idx = -1 in aux), so
    their stale SBUF content contributes nothing.
  - All compute is fp32-exact. Per tile:
      * VectorE, m-channels [0, MV): spatial j-reduce to xs[n, m] (2 halves),
        then TensorE accumulates psum_small[g, m] += w[n, g]^T @ xs[n, m].
      * TensorE, m-channels [MV, 64): fused segment-sum directly on raw x,
        psum_big[g, (m,j)] += w[n, g]^T @ x[n, (m,j)] in 4 psum chunks.
    with w the scale-weighted one-hot (scale = 1/(count_g*49)), generated ON
    DEVICE from a 74 KB aux tensor so the HBM stream is just x.
  - Tail pipelining: the last two tiles are DMA'd in pieces (vec halves
    first, PE chunks last; one semaphore PER PIECE — cumulative counts race
    across in-flight pieces because the 16 SDMA engines' +1s interleave).
    ScalarE copies psum_small -> out_sb (PSUM port, otherwise idle; its ACT
    Copy table is pre-warmed — the first activation pays a ~1.3us table
    load). VectorE j-reduces psum_big as the last matmul chunks complete.
    aux rides ScalarE's separate HWDGE queue so the x stream starts on the
    sync engine's first instruction.
  - PSUM rules honored: one accumulation start per BANK (start=True clears
    whole-bank accumulate bits); VectorE only reads a bank once no more
    matmuls will write it (same-bank PE-write + DVE-read is fatal).

Raw Block implementation (not Tile): the walrus matmul/DMA lowerings only
accept ONE attached sync-wait per instruction; standalone wait_ge
instructions sidestep that.
"""

from contextlib import ExitStack

import numpy as np

import concourse.bass as bass
import concourse.mybir as mybir
from concourse.bass_utils import run_bass_kernel_spmd

N = 2048          # samples
M = 512           # channels
HW = 49           # spatial (7*7)
G = 64            # groups
CORES = 8
ML = M // CORES   # 64 channels per core
F = ML * HW       # 3136 floats per (n, core)
P = 128           # partitions
BUFS = 8          # x-tile buffer depth

MV = 44           # m-channels through VectorE spatial-reduce + small matmul
MP = ML - MV      # 20 m-channels through TensorE raw fused matmul
FV = MV * HW      # 2156 vec-path columns
FP = MP * HW      # 980 raw columns through the PE
VH = (MV // 2) * HW  # 1078: half of the vec region (reduce granularity)

# --- tile schedule: de-load SDMA engine 15 (partitions 92-95, 124-127) ---
# partition ranges (dest) per tile kind; rows pack consecutively into ranges
FULL = ((0, 128),)
LITE = ((0, 92), (96, 124))
MINI = ((36, 64), (96, 100))
# NOTE: partition-subset DMAs measured catastrophically slow — the DGE uses
# the largest divisor <= 16 of the partition count as its engine set with
# port-MISALIGNED blocks (e.g. 92 partitions -> 4 engines, 120 -> 15 engines
# at ~70 ns/KB vs 44 aligned). Only exact-128-partition transfers hit the
# fast path, so every tile is FULL and engine 15 keeps its 1/16 share.
TILE_KINDS = [FULL] * 16
NTILES = len(TILE_KINDS)  # 16
TILE_ROWS = [sum(hi - lo for lo, hi in k) for k in TILE_KINDS]
assert sum(TILE_ROWS) == N
TILE_START = np.concatenate([[0], np.cumsum(TILE_ROWS)[:-1]]).tolist()

# fp32 matmul chunks within psum banks (bank 0 = cols [0:512), bank 1 =
# [512:980)). Only the first chunk per bank sets start=True at t=0.
CHUNKS = [(0, 256), (256, 512), (512, 768), (768, FP)]
NCH = len(CHUNKS)
CHUNK_START = {0, 2}

# epilogue j-reduce of psum_big: (0,10) reads bank 0 only -> after the last
# tile's chunk 1; (10,20) spans both banks -> after ALL matmuls.
SUBRED = [
    (0, 10, (NTILES - 1) * NCH + 2),
    (10, MP, NTILES * NCH),
]

# DMA pieces (column ranges) per tile; last two tiles split for the tail.
PIECES = {t: [(0, F)] for t in range(NTILES)}
PIECES[NTILES - 2] = [(0, VH), (VH, FV), (FV, F)]
PIECES[NTILES - 1] = [(0, VH), (VH, FV)] + \
    [(FV + lo, FV + hi) for lo, hi in CHUNKS]

F32 = mybir.dt.float32


def _build(wait_out=True):
    nc = bass.Bass(trn_type="TRN2", target_bir_lowering=False)
    x_ext = nc.declare_dram_parameter("x", [N, F], F32, isOutput=False)
    # aux[:, 0:64] iota row, aux[:, 64:128] scale row, aux[:, 128:] per-tile
    # local row index (or -1 for unused partitions)
    aux_ext = nc.declare_dram_parameter("aux", [P, G + G + NTILES], F32,
                                        isOutput=False)
    out_ext = nc.declare_dram_parameter("out", [G, ML], F32, isOutput=True)

    xr = x_ext.ap()  # [N, F]

    with ExitStack() as ctx:
        x_buf = ctx.enter_context(nc.sbuf_tensor([P, BUFS * F], F32))
        xs_buf = ctx.enter_context(nc.sbuf_tensor([P, BUFS * MV], F32))
        aux_sb = ctx.enter_context(nc.sbuf_tensor([P, G + G + NTILES], F32))
        warm_sb = ctx.enter_context(nc.sbuf_tensor([G, 2], F32))
        w_sb = ctx.enter_context(nc.sbuf_tensor([P, NTILES * G], F32))
        out_sb = ctx.enter_context(nc.sbuf_tensor([G, ML], F32))
        psum_big = ctx.enter_context(nc.psum_tensor([G, FP], F32))
        psum_small = ctx.enter_context(nc.psum_tensor([G, MV], F32))
        # one sem per (tile, piece): piece k of tile t complete at
        # 16 * n_partition_ranges
        dma_x = {
            t: [
                ctx.enter_context(nc.semaphore(name=f"dx{t}_{k}"))
                for k in range(len(PIECES[t]))
            ]
            for t in range(NTILES)
        }
        dma_a = ctx.enter_context(nc.semaphore())   # +16 when aux resident
        dma_o = ctx.enter_context(nc.semaphore())   # +16 when out written
        wg_sem = ctx.enter_context(nc.semaphore())  # +1 when w generated
        red_sem = ctx.enter_context(nc.semaphore())  # +2 per tile j-reduce
        pe_big = ctx.enter_context(nc.semaphore())   # +1 per big matmul chunk
        pe_tile = ctx.enter_context(nc.semaphore())  # +1 per tile (small mm)
        fin_sem = ctx.enter_context(nc.semaphore())  # +3 when out_sb ready
        block = ctx.enter_context(nc.Block())

        def piece_done(engine, t, k):
            engine.wait_ge(dma_x[t][k], 16 * len(TILE_KINDS[t]))

        def vec_wait(engine, t, half):
            piece_done(engine, t, half if len(PIECES[t]) > 1 else 0)

        def pe_wait(engine, t, chunk):
            if len(PIECES[t]) == 1:
                k = 0
            elif t == NTILES - 2:
                k = 2
            else:
                k = 2 + chunk
            piece_done(engine, t, k)

        # ---- DMA program for x + out (SP / HWDGE, FIFO) ----
        @block.sync
        def _(sync):
            for t in range(NTILES):
                if t >= BUFS:
                    # slot reuse: small matmul is ordered after the tile's
                    # j-reduces and big matmuls
                    sync.wait_ge(pe_tile, t - BUFS + 1)
                slot = t % BUFS
                row = TILE_START[t]
                for k, (lo, hi) in enumerate(PIECES[t]):
                    for plo, phi in TILE_KINDS[t]:
                        nrows = phi - plo
                        sync.dma_start(
                            out=x_buf[plo:phi, slot * F + lo:slot * F + hi],
                            in_=xr[row:row + nrows, lo:hi],
                        ).then_inc(dma_x[t][k], 16)
                        row += nrows
                    row -= TILE_ROWS[t]
            sync.wait_ge(fin_sem, 3)
            sync.dma_start(out=out_ext.ap(), in_=out_sb[:, :]).then_inc(dma_o, 16)
            if wait_out:
                sync.wait_ge(dma_o, 16)

        # ---- ScalarE: aux DMA on the second HWDGE queue; psum_small copy ----
        @block.scalar
        def _(scalar):
            scalar.dma_start(out=aux_sb[:, :], in_=aux_ext.ap()).then_inc(dma_a, 16)
            # warm the ACT Copy PWP table now — the first activation pays a
            # ~1.3us ACT_TABLE_LOAD which must not hit the final-copy path
            scalar.copy(warm_sb[:, 0:1], warm_sb[:, 1:2])
            scalar.wait_ge(pe_tile, NTILES)
            scalar.copy(out_sb[:, 0:MV], psum_small[:, :]).then_inc(fin_sem, 1)

        # ---- VectorE: w generation, spatial j-reduction, psum_big epilogue ----
        @block.vector
        def _(vector):
            # scale-weighted one-hot from the per-tile local row index:
            #   w[p, t*G+g] = (idx[tile t, partition p] == g) * scale[g]
            vector.wait_ge(dma_a, 16)
            for t in range(NTILES):
                wg = vector.scalar_tensor_tensor(
                    out=w_sb[:, t * G:(t + 1) * G],
                    in0=aux_sb[:, 0:G],
                    scalar=aux_sb[:, 2 * G + t:2 * G + t + 1],
                    in1=aux_sb[:, G:2 * G],
                    op0=mybir.AluOpType.is_equal,
                    op1=mybir.AluOpType.mult,
                )
            wg.then_inc(wg_sem, 1)

            for t in range(NTILES):
                slot = t % BUFS
                if t >= BUFS:
                    # xs slot reuse: wait until tile t-BUFS consumed by PE
                    vector.wait_ge(pe_tile, t - BUFS + 1)
                for half in range(2):
                    vec_wait(vector, t, half)
                    vector.tensor_reduce(
                        out=xs_buf[:, slot * MV + half * (MV // 2):
                                   slot * MV + (half + 1) * (MV // 2)],
                        in_=x_buf[:, slot * F + half * VH:
                                  slot * F + (half + 1) * VH].rearrange(
                            "p (m j) -> p m j", j=HW
                        ),
                        axis=mybir.AxisListType.X,
                        op=mybir.AluOpType.add,
                    ).then_inc(red_sem, 1)

            # epilogue: j-reduce psum_big as the last tile's chunks complete
            for mlo, mhi, need in SUBRED:
                vector.wait_ge(pe_big, need)
                vector.tensor_reduce(
                    out=out_sb[:, MV + mlo:MV + mhi],
                    in_=psum_big[:, mlo * HW:mhi * HW].rearrange(
                        "p (m j) -> p m j", j=HW
                    ),
                    axis=mybir.AxisListType.X,
                    op=mybir.AluOpType.add,
                ).then_inc(fin_sem, 1)

        # ---- TensorE: segment-sum accumulation (fp32) ----
        @block.tensor
        def _(tensor):
            tensor.wait_ge(wg_sem, 1)
            for t in range(NTILES):
                slot = t % BUFS
                wt = w_sb[:, t * G:(t + 1) * G]
                prev_key = None
                for c, (lo, hi) in enumerate(CHUNKS):
                    key = (t, 2 + c) if t == NTILES - 1 else (t, 0)
                    if key != prev_key:
                        pe_wait(tensor, t, c)
                        prev_key = key
                    tensor.matmul(
                        out=psum_big[:, lo:hi],
                        lhsT=wt,
                        rhs=x_buf[:, slot * F + FV + lo:slot * F + FV + hi],
                        start=(t == 0 and c in CHUNK_START),
                        stop=(t == NTILES - 1),
                        skip_group_check=True,
                    ).then_inc(pe_big, 1)
                tensor.wait_ge(red_sem, 2 * (t + 1))
                tensor.matmul(
                    out=psum_small[:, :],
                    lhsT=wt,
                    rhs=xs_buf[:, slot * MV:(slot + 1) * MV],
                    start=(t == 0),
                    stop=(t == NTILES - 1),
                ).then_inc(pe_tile, 1)

    return nc


def _prepare(x, idx):
    x = np.asarray(x)
    if x.dtype != np.float32:
        x = x.astype(np.float32)
    idx = np.asarray(idx).astype(np.int64)
    counts = np.bincount(idx, minlength=G).astype(np.float64)
    scale = np.where(counts > 0, 1.0 / (counts * HW), 0.0).astype(np.float32)
    aux = np.zeros((P, G + G + NTILES), np.float32)
    aux[:, 0:G] = np.arange(G, dtype=np.float32)[None, :]
    aux[:, G:2 * G] = scale[None, :]
    # per-tile: partition p holds row TILE_START[t] + local(p), or no row
    # (idx -1 -> w row of zeros kills the stale SBUF contents)
    for t in range(NTILES):
        col = np.full(P, -1.0, np.float32)
        row = TILE_START[t]
        for plo, phi in TILE_KINDS[t]:
            nrows = phi - plo
            col[plo:phi] = idx[row:row + nrows].astype(np.float32)
            row += nrows
        aux[:, 2 * G + t] = col
    xr = x.reshape(N, M, HW)
    in_maps = []
    for k in range(CORES):
        shard = np.ascontiguousarray(xr[:, k * ML:(k + 1) * ML, :]).reshape(N, F)
        in_maps.append({"x": shard, "aux": aux})
    return in_maps


def run(x, tensor_list_assignmentindices, trace=False, wait_out=True):
    in_maps = _prepare(x, tensor_list_assignmentindices)
    nc = _build(wait_out=wait_out)
    res = run_bass_kernel_spmd(nc, in_maps, core_ids=list(range(CORES)), trace=trace)
    outs = [np.asarray(r["out"]) for r in res.results]
    out = np.concatenate(outs, axis=1)  # [G, M]
    return out.reshape(G, M, 1, 1).astype(np.float32), res.exec_time_ns


def kernel(**inputs):
    out, _ = run(inputs["x"], inputs["tensor_list_assignmentindices"], trace=False)
    return out


# revision 11
# speedup vs baseline: 1.5343x; 1.0068x over previous
"""Trainium2 Bass kernel for nn_AvgPoolVectorsPerWSI (segment-mean over groups).

Math: x [N=2048, M=512, 7, 7], idx [N] in [0,64)
  out[g, m] = mean over {n: idx[n]==g} and spatial of x[n, m, :, :]  -> [64, 512, 1, 1]

Strategy (no collectives needed):
  - Shard over M: core k handles an m-slice of 64 channels. Each core reads
    its x slice [2048, 64, 49] (25.7 MB) once; the stream runs at the SDMA
    port limit (~26 GB/s/engine, 12.5 KB packets) -> ~62-65 us.
  - SDMA engine 15 intermittently runs ~20-45% slow (known HW issue) and
    would gate every transfer's completion. Engine k serves fixed SBUF
    partitions; engine 15 owns {92-95, 124-127}. So rows are packed into 17
    tiles: 12 FULL (128 rows, all partitions), 4 LITE (120 rows, partitions
    [0:92)+[96:124) -> engine 15 idle), 1 MINI (32 rows). Engine 15 carries
    96 of 2048 rows and the last two tiles never touch it, so a slow engine
    15 stops mattering. Unused partitions get w == 0 (idx = -1 in aux), so
    their stale SBUF content contributes nothing.
  - All compute is fp32-exact. Per tile:
      * VectorE, m-channels [0, MV): spatial j-reduce to xs[n, m] (2 halves),
        then TensorE accumulates psum_small[g, m] += w[n, g]^T @ xs[n, m].
      * TensorE, m-channels [MV, 64): fused segment-sum directly on raw x,
        psum_big[g, (m,j)] += w[n, g]^T @ x[n, (m,j)] in 4 psum chunks.
    with w the scale-weighted one-hot (scale = 1/(count_g*49)), generated ON
    DEVICE from a 74 KB aux tensor so the HBM stream is just x.
  - Tail pipelining: the last two tiles are DMA'd in pieces (vec halves
    first, PE chunks last; one semaphore PER PIECE — cumulative counts race
    across in-flight pieces because the 16 SDMA engines' +1s interleave).
    ScalarE copies psum_small -> out_sb (PSUM port, otherwise idle; its ACT
    Copy table is pre-warmed — the first activation pays a ~1.3us table
    load). VectorE j-reduces psum_big as the last matmul chunks complete.
    aux rides ScalarE's separate HWDGE queue so the x stream starts on the
    sync engine's first instruction.
  - PSUM rules honored: one accumulation start per BANK (start=True clears
    whole-bank accumulate bits); VectorE only reads a bank once no more
    matmuls will write it (same-bank PE-write + DVE-read is fatal).

Raw Block implementation (not Tile): the walrus matmul/DMA lowerings only
accept ONE attached sync-wait per instruction; standalone wait_ge
instructions sidestep that.
"""

from contextlib import ExitStack

import numpy as np

import concourse.bass as bass
import concourse.mybir as mybir
from concourse.bass_utils import run_bass_kernel_spmd

N = 2048          # samples
M = 512           # channels
HW = 49           # spatial (7*7)
G = 64            # groups
CORES = 8
ML = M // CORES   # 64 channels per core
F = ML * HW       # 3136 floats per (n, core)
P = 128           # partitions
BUFS = 8          # x-tile buffer depth

MV = 48           # m-channels through VectorE spatial-reduce + small matmul
MP = ML - MV      # 20 m-channels through TensorE raw fused matmul
FV = MV * HW      # 2156 vec-path columns
FP = MP * HW      # 980 raw columns through the PE
VH = (MV // 2) * HW  # 1078: half of the vec region (reduce granularity)

# --- tile schedule: de-load SDMA engine 15 (partitions 92-95, 124-127) ---
# partition ranges (dest) per tile kind; rows pack consecutively into ranges
FULL = ((0, 128),)
LITE = ((0, 92), (96, 124))
MINI = ((36, 64), (96, 100))
# NOTE: partition-subset DMAs measured catastrophically slow — the DGE uses
# the largest divisor <= 16 of the partition count as its engine set with
# port-MISALIGNED blocks (e.g. 92 partitions -> 4 engines, 120 -> 15 engines
# at ~70 ns/KB vs 44 aligned). Only exact-128-partition transfers hit the
# fast path, so every tile is FULL and engine 15 keeps its 1/16 share.
TILE_KINDS = [FULL] * 16
NTILES = len(TILE_KINDS)  # 16
TILE_ROWS = [sum(hi - lo for lo, hi in k) for k in TILE_KINDS]
assert sum(TILE_ROWS) == N
TILE_START = np.concatenate([[0], np.cumsum(TILE_ROWS)[:-1]]).tolist()

# fp32 matmul chunks within psum banks (bank 0 = cols [0:512), bank 1 =
# [512:784)). fp32 runs 2 PE passes per matmul and every pass pays a
# ~163ns LDWEIGHTS (ldw-opt is off in this toolchain), so tiles 0..14 use
# only 2 chunks; the last tile uses 3 so its matmuls pipeline against the
# arriving pieces. Only the first chunk per bank sets start=True at t=0
# (start clears the whole bank's accumulate bits).
CHUNKS = [(0, 512), (512, FP)]
CHUNKS_LAST = [(0, 256), (256, 512), (512, FP)]
CHUNK_START = {0, 1}      # by bank: chunk covering col 0 and col 512
NCH = len(CHUNKS)
NCHL = len(CHUNKS_LAST)
PE_BIG_TOTAL = (NTILES - 1) * NCH + NCHL

# epilogue j-reduce of psum_big: (0,10) reads bank 0 only -> after the last
# tile's bank-0 chunks; (10,MP) spans both banks -> after ALL matmuls.
SUBRED = [
    (0, 10, (NTILES - 1) * NCH + 2),
    (10, MP, PE_BIG_TOTAL),
]

# DMA pieces (column ranges) per tile; last two tiles split for the tail.
PIECES = {t: [(0, F)] for t in range(NTILES)}
PIECES[NTILES - 2] = [(0, VH), (VH, FV), (FV, F)]
PIECES[NTILES - 1] = [(0, VH), (VH, FV)] + \
    [(FV + lo, FV + hi) for lo, hi in CHUNKS_LAST]

F32 = mybir.dt.float32


def _build(wait_out=True):
    nc = bass.Bass(trn_type="TRN2", target_bir_lowering=False)
    x_ext = nc.declare_dram_parameter("x", [N, F], F32, isOutput=False)
    # aux[:, 0:64] iota row, aux[:, 64:128] scale row, aux[:, 128:] per-tile
    # local row index (or -1 for unused partitions)
    aux_ext = nc.declare_dram_parameter("aux", [P, G + G + NTILES], F32,
                                        isOutput=False)
    out_ext = nc.declare_dram_parameter("out", [G, ML], F32, isOutput=True)

    xr = x_ext.ap()  # [N, F]

    with ExitStack() as ctx:
        x_buf = ctx.enter_context(nc.sbuf_tensor([P, BUFS * F], F32))
        xs_buf = ctx.enter_context(nc.sbuf_tensor([P, BUFS * MV], F32))
        aux_sb = ctx.enter_context(nc.sbuf_tensor([P, G + G + NTILES], F32))
        warm_sb = ctx.enter_context(nc.sbuf_tensor([G, 2], F32))
        w_sb = ctx.enter_context(nc.sbuf_tensor([P, NTILES * G], F32))
        out_sb = ctx.enter_context(nc.sbuf_tensor([G, ML], F32))
        psum_big = ctx.enter_context(nc.psum_tensor([G, FP], F32))
        psum_small = ctx.enter_context(nc.psum_tensor([G, MV], F32))
        # one sem per (tile, piece): piece k of tile t complete at
        # 16 * n_partition_ranges
        dma_x = {
            t: [
                ctx.enter_context(nc.semaphore(name=f"dx{t}_{k}"))
                for k in range(len(PIECES[t]))
            ]
            for t in range(NTILES)
        }
        dma_a = ctx.enter_context(nc.semaphore())   # +16 when aux resident
        dma_o = ctx.enter_context(nc.semaphore())   # +16 when out written
        wg_sem = ctx.enter_context(nc.semaphore())  # +1 when w generated
        red_sem = ctx.enter_context(nc.semaphore())  # +2 per tile j-reduce
        pe_big = ctx.enter_context(nc.semaphore())   # +1 per big matmul chunk
        pe_tile = ctx.enter_context(nc.semaphore())  # +1 per tile (small mm)
        fin_sem = ctx.enter_context(nc.semaphore())  # +3 when out_sb ready
        block = ctx.enter_context(nc.Block())

        def piece_done(engine, t, k):
            engine.wait_ge(dma_x[t][k], 16 * len(TILE_KINDS[t]))

        def vec_wait(engine, t, half):
            piece_done(engine, t, half if len(PIECES[t]) > 1 else 0)

        def pe_wait(engine, t, chunk):
            if len(PIECES[t]) == 1:
                k = 0
            elif t == NTILES - 2:
                k = 2
            else:
                k = 2 + chunk
            piece_done(engine, t, k)

        # ---- DMA program for x + out (SP / HWDGE, FIFO) ----
        @block.sync
        def _(sync):
            for t in range(NTILES):
                if t >= BUFS:
                    # slot reuse: small matmul is ordered after the tile's
                    # j-reduces and big matmuls
                    sync.wait_ge(pe_tile, t - BUFS + 1)
                slot = t % BUFS
                row = TILE_START[t]
                for k, (lo, hi) in enumerate(PIECES[t]):
                    for plo, phi in TILE_KINDS[t]:
                        nrows = phi - plo
                        sync.dma_start(
                            out=x_buf[plo:phi, slot * F + lo:slot * F + hi],
                            in_=xr[row:row + nrows, lo:hi],
                        ).then_inc(dma_x[t][k], 16)
                        row += nrows
                    row -= TILE_ROWS[t]
            sync.wait_ge(fin_sem, 3)
            sync.dma_start(out=out_ext.ap(), in_=out_sb[:, :]).then_inc(dma_o, 16)
            if wait_out:
                sync.wait_ge(dma_o, 16)

        # ---- ScalarE: aux DMA on the second HWDGE queue; psum_small copy ----
        @block.scalar
        def _(scalar):
            scalar.dma_start(out=aux_sb[:, :], in_=aux_ext.ap()).then_inc(dma_a, 16)
            # warm the ACT Copy PWP table now — the first activation pays a
            # ~1.3us ACT_TABLE_LOAD which must not hit the final-copy path
            scalar.copy(warm_sb[:, 0:1], warm_sb[:, 1:2])
            scalar.wait_ge(pe_tile, NTILES)
            scalar.copy(out_sb[:, 0:MV], psum_small[:, :]).then_inc(fin_sem, 1)

        # ---- VectorE: w generation, spatial j-reduction, psum_big epilogue ----
        @block.vector
        def _(vector):
            # scale-weighted one-hot from the per-tile local row index:
            #   w[p, t*G+g] = (idx[tile t, partition p] == g) * scale[g]
            vector.wait_ge(dma_a, 16)
            for t in range(NTILES):
                wg = vector.scalar_tensor_tensor(
                    out=w_sb[:, t * G:(t + 1) * G],
                    in0=aux_sb[:, 0:G],
                    scalar=aux_sb[:, 2 * G + t:2 * G + t + 1],
                    in1=aux_sb[:, G:2 * G],
                    op0=mybir.AluOpType.is_equal,
                    op1=mybir.AluOpType.mult,
                )
            wg.then_inc(wg_sem, 1)

            for t in range(NTILES):
                slot = t % BUFS
                if t >= BUFS:
                    # xs slot reuse: wait until tile t-BUFS consumed by PE
                    vector.wait_ge(pe_tile, t - BUFS + 1)
                for half in range(2):
                    vec_wait(vector, t, half)
                    vector.tensor_reduce(
                        out=xs_buf[:, slot * MV + half * (MV // 2):
                                   slot * MV + (half + 1) * (MV // 2)],
                        in_=x_buf[:, slot * F + half * VH:
                                  slot * F + (half + 1) * VH].rearrange(
                            "p (m j) -> p m j", j=HW
                        ),
                        axis=mybir.AxisListType.X,
                        op=mybir.AluOpType.add,
                    ).then_inc(red_sem, 1)

            # epilogue: j-reduce psum_big as the last tile's chunks complete
            for mlo, mhi, need in SUBRED:
                vector.wait_ge(pe_big, need)
                vector.tensor_reduce(
                    out=out_sb[:, MV + mlo:MV + mhi],
                    in_=psum_big[:, mlo * HW:mhi * HW].rearrange(
                        "p (m j) -> p m j", j=HW
                    ),
                    axis=mybir.AxisListType.X,
                    op=mybir.AluOpType.add,
                ).then_inc(fin_sem, 1)

        # ---- TensorE: segment-sum accumulation (fp32) ----
        @block.tensor
        def _(tensor):
            tensor.wait_ge(wg_sem, 1)
            for t in range(NTILES):
                slot = t % BUFS
                wt = w_sb[:, t * G:(t + 1) * G]
                chunks = CHUNKS_LAST if t == NTILES - 1 else CHUNKS
                prev_key = None
                for c, (lo, hi) in enumerate(chunks):
                    key = (t, 2 + c) if t == NTILES - 1 else (t, 0)
                    if key != prev_key:
                        pe_wait(tensor, t, c)
                        prev_key = key
                    tensor.matmul(
                        out=psum_big[:, lo:hi],
                        lhsT=wt,
                        rhs=x_buf[:, slot * F + FV + lo:slot * F + FV + hi],
                        start=(t == 0 and lo in (0, 512)),
                        stop=(t == NTILES - 1),
                        skip_group_check=True,
                    ).then_inc(pe_big, 1)
                tensor.wait_ge(red_sem, 2 * (t + 1))
                tensor.matmul(
                    out=psum_small[:, :],
                    lhsT=wt,
                    rhs=xs_buf[:, slot * MV:(slot + 1) * MV],
                    start=(t == 0),
                    stop=(t == NTILES - 1),
                ).then_inc(pe_tile, 1)

    return nc


def _prepare(x, idx):
    x = np.asarray(x)
    if x.dtype != np.float32:
        x = x.astype(np.float32)
    idx = np.asarray(idx).astype(np.int64)
    counts = np.bincount(idx, minlength=G).astype(np.float64)
    scale = np.where(counts > 0, 1.0 / (counts * HW), 0.0).astype(np.float32)
    aux = np.zeros((P, G + G + NTILES), np.float32)
    aux[:, 0:G] = np.arange(G, dtype=np.float32)[None, :]
    aux[:, G:2 * G] = scale[None, :]
    # per-tile: partition p holds row TILE_START[t] + local(p), or no row
    # (idx -1 -> w row of zeros kills the stale SBUF contents)
    for t in range(NTILES):
        col = np.full(P, -1.0, np.float32)
        row = TILE_START[t]
        for plo, phi in TILE_KINDS[t]:
            nrows = phi - plo
            col[plo:phi] = idx[row:row + nrows].astype(np.float32)
            row += nrows
        aux[:, 2 * G + t] = col
    xr = x.reshape(N, M, HW)
    in_maps = []
    for k in range(CORES):
        shard = np.ascontiguousarray(xr[:, k * ML:(k + 1) * ML, :]).reshape(N, F)
        in_maps.append({"x": shard, "aux": aux})
    return in_maps


def run(x, tensor_list_assignmentindices, trace=False, wait_out=True):
    in_maps = _prepare(x, tensor_list_assignmentindices)
    nc = _build(wait_out=wait_out)
    res = run_bass_kernel_spmd(nc, in_maps, core_ids=list(range(CORES)), trace=trace)
    outs = [np.asarray(r["out"]) for r in res.results]
    out = np.concatenate(outs, axis=1)  # [G, M]
    return out.reshape(G, M, 1, 1).astype(np.float32), res.exec_time_ns


def kernel(**inputs):
    out, _ = run(inputs["x"], inputs["tensor_list_assignmentindices"], trace=False)
    return out


# revision 12
# speedup vs baseline: 1.6884x; 1.1004x over previous
"""Trainium2 Bass kernel for nn_AvgPoolVectorsPerWSI (segment-mean over groups).

Math: x [N=2048, M=512, 7, 7], idx [N] in [0,64)
  out[g, m] = mean over {n: idx[n]==g} and spatial of x[n, m, :, :]  -> [64, 512, 1, 1]

Strategy (no collectives needed):
  - Shard over M: core k handles an m-slice of 64 channels. Each core reads
    its x slice [2048, 64, 49] (25.7 MB) once; the stream runs at the SDMA
    port limit (~26 GB/s/engine, 12.5 KB packets) -> ~62-65 us.
  - SDMA engine 15 intermittently runs ~20-45% slow (known HW issue) and
    would gate every transfer's completion. Engine k serves fixed SBUF
    partitions; engine 15 owns {92-95, 124-127}. So rows are packed into 17
    tiles: 12 FULL (128 rows, all partitions), 4 LITE (120 rows, partitions
    [0:92)+[96:124) -> engine 15 idle), 1 MINI (32 rows). Engine 15 carries
    96 of 2048 rows and the last two tiles never touch it, so a slow engine
    15 stops mattering. Unused partitions get w == 0 (idx = -1 in aux), so
    their stale SBUF content contributes nothing.
  - All compute is fp32-exact. Per tile:
      * VectorE, m-channels [0, MV): spatial j-reduce to xs[n, m] (2 halves),
        then TensorE accumulates psum_small[g, m] += w[n, g]^T @ xs[n, m].
      * TensorE, m-channels [MV, 64): fused segment-sum directly on raw x,
        psum_big[g, (m,j)] += w[n, g]^T @ x[n, (m,j)] in 4 psum chunks.
    with w the scale-weighted one-hot (scale = 1/(count_g*49)), generated ON
    DEVICE from a 74 KB aux tensor so the HBM stream is just x.
  - Tail pipelining: the last two tiles are DMA'd in pieces (vec halves
    first, PE chunks last; one semaphore PER PIECE — cumulative counts race
    across in-flight pieces because the 16 SDMA engines' +1s interleave).
    ScalarE copies psum_small -> out_sb (PSUM port, otherwise idle; its ACT
    Copy table is pre-warmed — the first activation pays a ~1.3us table
    load). VectorE j-reduces psum_big as the last matmul chunks complete.
    aux rides ScalarE's separate HWDGE queue so the x stream starts on the
    sync engine's first instruction.
  - PSUM rules honored: one accumulation start per BANK (start=True clears
    whole-bank accumulate bits); VectorE only reads a bank once no more
    matmuls will write it (same-bank PE-write + DVE-read is fatal).

Raw Block implementation (not Tile): the walrus matmul/DMA lowerings only
accept ONE attached sync-wait per instruction; standalone wait_ge
instructions sidestep that.
"""

from contextlib import ExitStack

import numpy as np

import concourse.bass as bass
import concourse.mybir as mybir
from concourse.bass_utils import run_bass_kernel_spmd

N = 2048          # samples
M = 512           # channels
HW = 49           # spatial (7*7)
G = 64            # groups
CORES = 8
ML = M // CORES   # 64 channels per core
F = ML * HW       # 3136 floats per (n, core)
P = 128           # partitions
BUFS = 8          # x-tile buffer depth

MV = 48           # m-channels through VectorE spatial-reduce + small matmul
MP = ML - MV      # 20 m-channels through TensorE raw fused matmul
FV = MV * HW      # 2156 vec-path columns
FP = MP * HW      # 980 raw columns through the PE
VH = (MV // 2) * HW  # 1078: half of the vec region (reduce granularity)

# --- tile schedule: de-load SDMA engine 15 (partitions 92-95, 124-127) ---
# partition ranges (dest) per tile kind; rows pack consecutively into ranges
FULL = ((0, 128),)
LITE = ((0, 92), (96, 124))
MINI = ((36, 64), (96, 100))
# NOTE: partition-subset DMAs measured catastrophically slow — the DGE uses
# the largest divisor <= 16 of the partition count as its engine set with
# port-MISALIGNED blocks (e.g. 92 partitions -> 4 engines, 120 -> 15 engines
# at ~70 ns/KB vs 44 aligned). Only exact-128-partition transfers hit the
# fast path, so every tile is FULL and engine 15 keeps its 1/16 share.
TILE_KINDS = [FULL] * 16
NTILES = len(TILE_KINDS)  # 16
TILE_ROWS = [sum(hi - lo for lo, hi in k) for k in TILE_KINDS]
assert sum(TILE_ROWS) == N
TILE_START = np.concatenate([[0], np.cumsum(TILE_ROWS)[:-1]]).tolist()

# fp32 matmul chunks within psum banks (bank 0 = cols [0:512), bank 1 =
# [512:784)). fp32 runs 2 PE passes per matmul and every pass pays a
# ~163ns LDWEIGHTS (ldw-opt is off in this toolchain), so tiles 0..14 use
# only 2 chunks; the last tile uses 3 so its matmuls pipeline against the
# arriving pieces. Only the first chunk per bank sets start=True at t=0
# (start clears the whole bank's accumulate bits).
CHUNKS = [(0, 512), (512, FP)]
CHUNKS_LAST = [(0, 256), (256, 512), (512, FP)]
CHUNK_START = {0, 1}      # by bank: chunk covering col 0 and col 512
NCH = len(CHUNKS)
NCHL = len(CHUNKS_LAST)
PE_BIG_TOTAL = (NTILES - 1) * NCH + NCHL

# epilogue j-reduce of psum_big: (0,10) reads bank 0 only -> after the last
# tile's bank-0 chunks; (10,MP) spans both banks -> after ALL matmuls.
SUBRED = [
    (0, 10, (NTILES - 1) * NCH + 2),
    (10, MP, PE_BIG_TOTAL),
]

VQ = (MV // 4) * HW  # 588: quarter of the vec region (last-tile granularity)
LAST = NTILES - 1

# DMA pieces (column ranges) per tile. Tile 0 splits so the first descriptor
# batch is short (earlier first byte); tiles 14/15 split so the tail
# pipelines; tile 15 interleaves vec quarters with PE chunk pieces so the PE
# starts its last chunks ~1.5us before the final byte lands.
PIECES = {t: [(0, F)] for t in range(NTILES)}
PIECES[0] = [(0, VH), (VH, F)]
PIECES[NTILES - 2] = [(0, VH), (VH, FV), (FV, F)]
PIECES[LAST] = [
    (0, VQ), (VQ, 2 * VQ),
    (FV, FV + 256),
    (2 * VQ, 3 * VQ),
    (FV + 256, FV + 512),
    (3 * VQ, FV),
    (FV + 512, F),
]

# per-tile vector reduces: (x col lo, hi, gating piece idx)
REDUCES = {}
for _t in range(NTILES):
    if _t == 0:
        REDUCES[_t] = [(0, VH, 0), (VH, FV, 1)]
    elif _t == NTILES - 2:
        REDUCES[_t] = [(0, VH, 0), (VH, FV, 1)]
    elif _t == LAST:
        REDUCES[_t] = [(0, VQ, 0), (VQ, 2 * VQ, 1), (2 * VQ, 3 * VQ, 3),
                       (3 * VQ, FV, 5)]
    else:
        REDUCES[_t] = [(0, VH, 0), (VH, FV, 0)]
RED_CUM = np.cumsum([len(REDUCES[_t]) for _t in range(NTILES)]).tolist()

# per-tile PE chunks: (psum col lo, hi, gating piece idx)
PE_CHUNKS = {}
for _t in range(NTILES):
    if _t == 0:
        PE_CHUNKS[_t] = [(lo, hi, 1) for lo, hi in CHUNKS]
    elif _t == NTILES - 2:
        PE_CHUNKS[_t] = [(lo, hi, 2) for lo, hi in CHUNKS]
    elif _t == LAST:
        PE_CHUNKS[_t] = [(CHUNKS_LAST[0][0], CHUNKS_LAST[0][1], 2),
                         (CHUNKS_LAST[1][0], CHUNKS_LAST[1][1], 4),
                         (CHUNKS_LAST[2][0], CHUNKS_LAST[2][1], 6)]
    else:
        PE_CHUNKS[_t] = [(lo, hi, 0) for lo, hi in CHUNKS]

F32 = mybir.dt.float32


def _build(wait_out=True):
    nc = bass.Bass(trn_type="TRN2", target_bir_lowering=False)
    x_ext = nc.declare_dram_parameter("x", [N, F], F32, isOutput=False)
    # aux[:, 0:64] iota row, aux[:, 64:128] scale row, aux[:, 128:] per-tile
    # local row index (or -1 for unused partitions)
    aux_ext = nc.declare_dram_parameter("aux", [P, G + G + NTILES], F32,
                                        isOutput=False)
    out_ext = nc.declare_dram_parameter("out", [G, ML], F32, isOutput=True)

    xr = x_ext.ap()  # [N, F]

    with ExitStack() as ctx:
        x_buf = ctx.enter_context(nc.sbuf_tensor([P, BUFS * F], F32))
        xs_buf = ctx.enter_context(nc.sbuf_tensor([P, BUFS * MV], F32))
        aux_sb = ctx.enter_context(nc.sbuf_tensor([P, G + G + NTILES], F32))
        warm_sb = ctx.enter_context(nc.sbuf_tensor([G, 2], F32))
        w_sb = ctx.enter_context(nc.sbuf_tensor([P, NTILES * G], F32))
        out_sb = ctx.enter_context(nc.sbuf_tensor([G, ML], F32))
        psum_big = ctx.enter_context(nc.psum_tensor([G, FP], F32))
        psum_small = ctx.enter_context(nc.psum_tensor([G, MV], F32))
        # one sem per (tile, piece): piece k of tile t complete at
        # 16 * n_partition_ranges
        dma_x = {
            t: [
                ctx.enter_context(nc.semaphore(name=f"dx{t}_{k}"))
                for k in range(len(PIECES[t]))
            ]
            for t in range(NTILES)
        }
        dma_a = ctx.enter_context(nc.semaphore())   # +16 when aux resident
        dma_o = ctx.enter_context(nc.semaphore())   # +16 when out written
        wg_sem = ctx.enter_context(nc.semaphore())  # +1 when w generated
        red_sem = ctx.enter_context(nc.semaphore())  # +2 per tile j-reduce
        pe_big = ctx.enter_context(nc.semaphore())   # +1 per big matmul chunk
        pe_tile = ctx.enter_context(nc.semaphore())  # +1 per tile (small mm)
        fin_sem = ctx.enter_context(nc.semaphore())  # +3 when out_sb ready
        block = ctx.enter_context(nc.Block())

        def piece_done(engine, t, k):
            engine.wait_ge(dma_x[t][k], 16 * len(TILE_KINDS[t]))

        # ---- DMA program for x + out (SP / HWDGE, FIFO) ----
        @block.sync
        def _(sync):
            for t in range(NTILES):
                if t >= BUFS:
                    # slot reuse: small matmul is ordered after the tile's
                    # j-reduces and big matmuls
                    sync.wait_ge(pe_tile, t - BUFS + 1)
                slot = t % BUFS
                row = TILE_START[t]
                for k, (lo, hi) in enumerate(PIECES[t]):
                    for plo, phi in TILE_KINDS[t]:
                        nrows = phi - plo
                        sync.dma_start(
                            out=x_buf[plo:phi, slot * F + lo:slot * F + hi],
                            in_=xr[row:row + nrows, lo:hi],
                        ).then_inc(dma_x[t][k], 16)
                        row += nrows
                    row -= TILE_ROWS[t]
            sync.wait_ge(fin_sem, 3)
            sync.dma_start(out=out_ext.ap(), in_=out_sb[:, :]).then_inc(dma_o, 16)
            if wait_out:
                sync.wait_ge(dma_o, 16)

        # ---- ScalarE: aux DMA on the second HWDGE queue; psum_small copy ----
        @block.scalar
        def _(scalar):
            scalar.dma_start(out=aux_sb[:, :], in_=aux_ext.ap()).then_inc(dma_a, 16)
            # warm the ACT Copy PWP table now — the first activation pays a
            # ~1.3us ACT_TABLE_LOAD which must not hit the final-copy path
            scalar.copy(warm_sb[:, 0:1], warm_sb[:, 1:2])
            scalar.wait_ge(pe_tile, NTILES)
            scalar.copy(out_sb[:, 0:MV], psum_small[:, :]).then_inc(fin_sem, 1)

        # ---- VectorE: w generation, spatial j-reduction, psum_big epilogue ----
        @block.vector
        def _(vector):
            # scale-weighted one-hot from the per-tile local row index:
            #   w[p, t*G+g] = (idx[tile t, partition p] == g) * scale[g]
            vector.wait_ge(dma_a, 16)
            for t in range(NTILES):
                wg = vector.scalar_tensor_tensor(
                    out=w_sb[:, t * G:(t + 1) * G],
                    in0=aux_sb[:, 0:G],
                    scalar=aux_sb[:, 2 * G + t:2 * G + t + 1],
                    in1=aux_sb[:, G:2 * G],
                    op0=mybir.AluOpType.is_equal,
                    op1=mybir.AluOpType.mult,
                )
            wg.then_inc(wg_sem, 1)

            for t in range(NTILES):
                slot = t % BUFS
                if t >= BUFS:
                    # xs slot reuse: wait until tile t-BUFS consumed by PE
                    vector.wait_ge(pe_tile, t - BUFS + 1)
                for lo, hi, pk in REDUCES[t]:
                    piece_done(vector, t, pk)
                    vector.tensor_reduce(
                        out=xs_buf[:, slot * MV + lo // HW:
                                   slot * MV + hi // HW],
                        in_=x_buf[:, slot * F + lo:slot * F + hi].rearrange(
                            "p (m j) -> p m j", j=HW
                        ),
                        axis=mybir.AxisListType.X,
                        op=mybir.AluOpType.add,
                    ).then_inc(red_sem, 1)

            # epilogue: j-reduce psum_big as the last tile's chunks complete
            for mlo, mhi, need in SUBRED:
                vector.wait_ge(pe_big, need)
                vector.tensor_reduce(
                    out=out_sb[:, MV + mlo:MV + mhi],
                    in_=psum_big[:, mlo * HW:mhi * HW].rearrange(
                        "p (m j) -> p m j", j=HW
                    ),
                    axis=mybir.AxisListType.X,
                    op=mybir.AluOpType.add,
                ).then_inc(fin_sem, 1)

        # ---- TensorE: segment-sum accumulation (fp32) ----
        @block.tensor
        def _(tensor):
            tensor.wait_ge(wg_sem, 1)
            for t in range(NTILES):
                slot = t % BUFS
                wt = w_sb[:, t * G:(t + 1) * G]
                prev_pk = None
                for lo, hi, pk in PE_CHUNKS[t]:
                    if pk != prev_pk:
                        piece_done(tensor, t, pk)
                        prev_pk = pk
                    tensor.matmul(
                        out=psum_big[:, lo:hi],
                        lhsT=wt,
                        rhs=x_buf[:, slot * F + FV + lo:slot * F + FV + hi],
                        start=(t == 0 and lo in (0, 512)),
                        stop=(t == NTILES - 1),
                        skip_group_check=True,
                    ).then_inc(pe_big, 1)
                tensor.wait_ge(red_sem, RED_CUM[t])
                tensor.matmul(
                    out=psum_small[:, :],
                    lhsT=wt,
                    rhs=xs_buf[:, slot * MV:(slot + 1) * MV],
                    start=(t == 0),
                    stop=(t == NTILES - 1),
                ).then_inc(pe_tile, 1)

    return nc


def _prepare(x, idx):
    x = np.asarray(x)
    if x.dtype != np.float32:
        x = x.astype(np.float32)
    idx = np.asarray(idx).astype(np.int64)
    counts = np.bincount(idx, minlength=G).astype(np.float64)
    scale = np.where(counts > 0, 1.0 / (counts * HW), 0.0).astype(np.float32)
    aux = np.zeros((P, G + G + NTILES), np.float32)
    aux[:, 0:G] = np.arange(G, dtype=np.float32)[None, :]
    aux[:, G:2 * G] = scale[None, :]
    # per-tile: partition p holds row TILE_START[t] + local(p), or no row
    # (idx -1 -> w row of zeros kills the stale SBUF contents)
    for t in range(NTILES):
        col = np.full(P, -1.0, np.float32)
        row = TILE_START[t]
        for plo, phi in TILE_KINDS[t]:
            nrows = phi - plo
            col[plo:phi] = idx[row:row + nrows].astype(np.float32)
            row += nrows
        aux[:, 2 * G + t] = col
    xr = x.reshape(N, M, HW)
    in_maps = []
    for k in range(CORES):
        shard = np.ascontiguousarray(xr[:, k * ML:(k + 1) * ML, :]).reshape(N, F)
        in_maps.append({"x": shard, "aux": aux})
    return in_maps


def run(x, tensor_list_assignmentindices, trace=False, wait_out=True):
    in_maps = _prepare(x, tensor_list_assignmentindices)
    nc = _build(wait_out=wait_out)
    res = run_bass_kernel_spmd(nc, in_maps, core_ids=list(range(CORES)), trace=trace)
    outs = [np.asarray(r["out"]) for r in res.results]
    out = np.concatenate(outs, axis=1)  # [G, M]
    return out.reshape(G, M, 1, 1).astype(np.float32), res.exec_time_ns


def kernel(**inputs):
    out, _ = run(inputs["x"], inputs["tensor_list_assignmentindices"], trace=False)
    return out


# revision 13
# speedup vs baseline: 1.7019x; 1.0080x over previous
"""Trainium2 Bass kernel for nn_AvgPoolVectorsPerWSI (segment-mean over groups).

Math: x [N=2048, M=512, 7, 7], idx [N] in [0,64)
  out[g, m] = mean over {n: idx[n]==g} and spatial of x[n, m, :, :]  -> [64, 512, 1, 1]

Strategy (no collectives needed):
  - Shard over M: core k handles an m-slice of 64 channels. Each core reads
    its x slice [2048, 64, 49] (25.7 MB) once; the stream runs at the SDMA
    port limit (~26 GB/s/engine, 12.5 KB packets) -> ~62-65 us.
  - SDMA engine 15 intermittently runs ~20-45% slow (known HW issue) and
    would gate every transfer's completion. Engine k serves fixed SBUF
    partitions; engine 15 owns {92-95, 124-127}. So rows are packed into 17
    tiles: 12 FULL (128 rows, all partitions), 4 LITE (120 rows, partitions
    [0:92)+[96:124) -> engine 15 idle), 1 MINI (32 rows). Engine 15 carries
    96 of 2048 rows and the last two tiles never touch it, so a slow engine
    15 stops mattering. Unused partitions get w == 0 (idx = -1 in aux), so
    their stale SBUF content contributes nothing.
  - All compute is fp32-exact. Per tile:
      * VectorE, m-channels [0, MV): spatial j-reduce to xs[n, m] (2 halves),
        then TensorE accumulates psum_small[g, m] += w[n, g]^T @ xs[n, m].
      * TensorE, m-channels [MV, 64): fused segment-sum directly on raw x,
        psum_big[g, (m,j)] += w[n, g]^T @ x[n, (m,j)] in 4 psum chunks.
    with w the scale-weighted one-hot (scale = 1/(count_g*49)), generated ON
    DEVICE from a 74 KB aux tensor so the HBM stream is just x.
  - Tail pipelining: the last two tiles are DMA'd in pieces (vec halves
    first, PE chunks last; one semaphore PER PIECE — cumulative counts race
    across in-flight pieces because the 16 SDMA engines' +1s interleave).
    ScalarE copies psum_small -> out_sb (PSUM port, otherwise idle; its ACT
    Copy table is pre-warmed — the first activation pays a ~1.3us table
    load). VectorE j-reduces psum_big as the last matmul chunks complete.
    aux rides ScalarE's separate HWDGE queue so the x stream starts on the
    sync engine's first instruction.
  - PSUM rules honored: one accumulation start per BANK (start=True clears
    whole-bank accumulate bits); VectorE only reads a bank once no more
    matmuls will write it (same-bank PE-write + DVE-read is fatal).

Raw Block implementation (not Tile): the walrus matmul/DMA lowerings only
accept ONE attached sync-wait per instruction; standalone wait_ge
instructions sidestep that.
"""

from contextlib import ExitStack

import numpy as np

import concourse.bass as bass
import concourse.mybir as mybir
from concourse.bass_utils import run_bass_kernel_spmd

N = 2048          # samples
M = 512           # channels
HW = 49           # spatial (7*7)
G = 64            # groups
CORES = 8
ML = M // CORES   # 64 channels per core
F = ML * HW       # 3136 floats per (n, core)
P = 128           # partitions
BUFS = 8          # x-tile buffer depth

MV = 48           # m-channels through VectorE spatial-reduce + small matmul
MP = ML - MV      # 20 m-channels through TensorE raw fused matmul
FV = MV * HW      # 2156 vec-path columns
FP = MP * HW      # 980 raw columns through the PE
VH = (MV // 2) * HW  # 1078: half of the vec region (reduce granularity)

# --- tile schedule: de-load SDMA engine 15 (partitions 92-95, 124-127) ---
# partition ranges (dest) per tile kind; rows pack consecutively into ranges
FULL = ((0, 128),)
LITE = ((0, 92), (96, 124))
MINI = ((36, 64), (96, 100))
# NOTE: partition-subset DMAs measured catastrophically slow — the DGE uses
# the largest divisor <= 16 of the partition count as its engine set with
# port-MISALIGNED blocks (e.g. 92 partitions -> 4 engines, 120 -> 15 engines
# at ~70 ns/KB vs 44 aligned). Only exact-128-partition transfers hit the
# fast path, so every tile is FULL and engine 15 keeps its 1/16 share.
TILE_KINDS = [FULL] * 16
NTILES = len(TILE_KINDS)  # 16
TILE_ROWS = [sum(hi - lo for lo, hi in k) for k in TILE_KINDS]
assert sum(TILE_ROWS) == N
TILE_START = np.concatenate([[0], np.cumsum(TILE_ROWS)[:-1]]).tolist()

# fp32 matmul chunks within psum banks (bank 0 = cols [0:512), bank 1 =
# [512:784)). fp32 runs 2 PE passes per matmul and every pass pays a
# ~163ns LDWEIGHTS (ldw-opt is off in this toolchain), so tiles 0..14 use
# only 2 chunks; the last tile uses 3 so its matmuls pipeline against the
# arriving pieces. Only the first chunk per bank sets start=True at t=0
# (start clears the whole bank's accumulate bits).
CHUNKS = [(0, 512), (512, FP)]
CHUNKS_LAST = [(0, 256), (256, 512), (512, FP)]
CHUNK_START = {0, 1}      # by bank: chunk covering col 0 and col 512
NCH = len(CHUNKS)
NCHL = len(CHUNKS_LAST)
PE_BIG_TOTAL = (NTILES - 1) * NCH + NCHL

# epilogue j-reduce of psum_big: (0,10) reads bank 0 only -> after the last
# tile's bank-0 chunks; (10,MP) spans both banks -> after ALL matmuls.
SUBRED = [
    (0, 10, (NTILES - 1) * NCH + 2),
    (10, MP, PE_BIG_TOTAL),
]

VQ = (MV // 4) * HW  # 588: quarter of the vec region (last-tile granularity)
LAST = NTILES - 1

# DMA pieces (column ranges) per tile. Tile 0 splits so the first descriptor
# batch is short (earlier first byte); tiles 14/15 split so the tail
# pipelines; tile 15 interleaves vec quarters with PE chunk pieces so the PE
# starts its last chunks ~1.5us before the final byte lands.
PIECES = {t: [(0, F)] for t in range(NTILES)}
PIECES[NTILES - 2] = [(0, VH), (VH, FV), (FV, F)]
PIECES[LAST] = [
    (0, VQ), (VQ, 2 * VQ),
    (FV, FV + 256),
    (2 * VQ, 3 * VQ),
    (FV + 256, FV + 512),
    (3 * VQ, FV),
    (FV + 512, F),
]

# per-tile vector reduces: (x col lo, hi, gating piece idx)
REDUCES = {}
for _t in range(NTILES):
    if _t == NTILES - 2:
        REDUCES[_t] = [(0, VH, 0), (VH, FV, 1)]
    elif _t == LAST:
        REDUCES[_t] = [(0, VQ, 0), (VQ, 2 * VQ, 1), (2 * VQ, 3 * VQ, 3),
                       (3 * VQ, FV, 5)]
    else:
        REDUCES[_t] = [(0, VH, 0), (VH, FV, 0)]
RED_CUM = np.cumsum([len(REDUCES[_t]) for _t in range(NTILES)]).tolist()

# per-tile PE chunks: (psum col lo, hi, gating piece idx)
PE_CHUNKS = {}
for _t in range(NTILES):
    if _t == NTILES - 2:
        PE_CHUNKS[_t] = [(lo, hi, 2) for lo, hi in CHUNKS]
    elif _t == LAST:
        PE_CHUNKS[_t] = [(CHUNKS_LAST[0][0], CHUNKS_LAST[0][1], 2),
                         (CHUNKS_LAST[1][0], CHUNKS_LAST[1][1], 4),
                         (CHUNKS_LAST[2][0], CHUNKS_LAST[2][1], 6)]
    else:
        PE_CHUNKS[_t] = [(lo, hi, 0) for lo, hi in CHUNKS]

F32 = mybir.dt.float32


def _build(wait_out=True):
    nc = bass.Bass(trn_type="TRN2", target_bir_lowering=False)
    x_ext = nc.declare_dram_parameter("x", [N, F], F32, isOutput=False)
    # aux[:, 0:64] iota row, aux[:, 64:128] scale row, aux[:, 128:] per-tile
    # local row index (or -1 for unused partitions)
    aux_ext = nc.declare_dram_parameter("aux", [P, G + G + NTILES], F32,
                                        isOutput=False)
    out_ext = nc.declare_dram_parameter("out", [G, ML], F32, isOutput=True)

    xr = x_ext.ap()  # [N, F]

    with ExitStack() as ctx:
        x_buf = ctx.enter_context(nc.sbuf_tensor([P, BUFS * F], F32))
        xs_buf = ctx.enter_context(nc.sbuf_tensor([P, BUFS * MV], F32))
        aux_sb = ctx.enter_context(nc.sbuf_tensor([P, G + G + NTILES], F32))
        warm_sb = ctx.enter_context(nc.sbuf_tensor([G, 2], F32))
        w_sb = ctx.enter_context(nc.sbuf_tensor([P, NTILES * G], F32))
        out_sb = ctx.enter_context(nc.sbuf_tensor([G, ML], F32))
        psum_big = ctx.enter_context(nc.psum_tensor([G, FP], F32))
        psum_small = ctx.enter_context(nc.psum_tensor([G, MV], F32))
        # one sem per (tile, piece): piece k of tile t complete at
        # 16 * n_partition_ranges
        dma_x = {
            t: [
                ctx.enter_context(nc.semaphore(name=f"dx{t}_{k}"))
                for k in range(len(PIECES[t]))
            ]
            for t in range(NTILES)
        }
        dma_a = ctx.enter_context(nc.semaphore())   # +16 when aux resident
        dma_o = ctx.enter_context(nc.semaphore())   # +16 when out written
        wg_sem = ctx.enter_context(nc.semaphore())  # +1 when w generated
        red_sem = ctx.enter_context(nc.semaphore())  # +2 per tile j-reduce
        pe_big = ctx.enter_context(nc.semaphore())   # +1 per big matmul chunk
        pe_tile = ctx.enter_context(nc.semaphore())  # +1 per tile (small mm)
        fin_sem = ctx.enter_context(nc.semaphore())  # +3 when out_sb ready
        block = ctx.enter_context(nc.Block())

        def piece_done(engine, t, k):
            engine.wait_ge(dma_x[t][k], 16 * len(TILE_KINDS[t]))

        # ---- DMA program for x + out (SP / HWDGE, FIFO) ----
        @block.sync
        def _(sync):
            for t in range(NTILES):
                if t >= BUFS:
                    # slot reuse: small matmul is ordered after the tile's
                    # j-reduces and big matmuls
                    sync.wait_ge(pe_tile, t - BUFS + 1)
                slot = t % BUFS
                row = TILE_START[t]
                for k, (lo, hi) in enumerate(PIECES[t]):
                    for plo, phi in TILE_KINDS[t]:
                        nrows = phi - plo
                        sync.dma_start(
                            out=x_buf[plo:phi, slot * F + lo:slot * F + hi],
                            in_=xr[row:row + nrows, lo:hi],
                        ).then_inc(dma_x[t][k], 16)
                        row += nrows
                    row -= TILE_ROWS[t]
            sync.wait_ge(fin_sem, 3)
            sync.dma_start(out=out_ext.ap(), in_=out_sb[:, :]).then_inc(dma_o, 16)
            if wait_out:
                sync.wait_ge(dma_o, 16)

        # ---- ScalarE: aux DMA on the second HWDGE queue; psum_small copy ----
        @block.scalar
        def _(scalar):
            scalar.dma_start(out=aux_sb[:, :], in_=aux_ext.ap()).then_inc(dma_a, 16)
            # warm the ACT Copy PWP table now — the first activation pays a
            # ~1.3us ACT_TABLE_LOAD which must not hit the final-copy path
            scalar.copy(warm_sb[:, 0:1], warm_sb[:, 1:2])
            scalar.wait_ge(pe_tile, NTILES)
            scalar.copy(out_sb[:, 0:MV], psum_small[:, :]).then_inc(fin_sem, 1)

        # ---- VectorE: w generation, spatial j-reduction, psum_big epilogue ----
        @block.vector
        def _(vector):
            # scale-weighted one-hot from the per-tile local row index:
            #   w[p, t*G+g] = (idx[tile t, partition p] == g) * scale[g]
            vector.wait_ge(dma_a, 16)
            for t in range(NTILES):
                wg = vector.scalar_tensor_tensor(
                    out=w_sb[:, t * G:(t + 1) * G],
                    in0=aux_sb[:, 0:G],
                    scalar=aux_sb[:, 2 * G + t:2 * G + t + 1],
                    in1=aux_sb[:, G:2 * G],
                    op0=mybir.AluOpType.is_equal,
                    op1=mybir.AluOpType.mult,
                )
            wg.then_inc(wg_sem, 1)

            for t in range(NTILES):
                slot = t % BUFS
                if t >= BUFS:
                    # xs slot reuse: wait until tile t-BUFS consumed by PE
                    vector.wait_ge(pe_tile, t - BUFS + 1)
                for lo, hi, pk in REDUCES[t]:
                    piece_done(vector, t, pk)
                    vector.tensor_reduce(
                        out=xs_buf[:, slot * MV + lo // HW:
                                   slot * MV + hi // HW],
                        in_=x_buf[:, slot * F + lo:slot * F + hi].rearrange(
                            "p (m j) -> p m j", j=HW
                        ),
                        axis=mybir.AxisListType.X,
                        op=mybir.AluOpType.add,
                    ).then_inc(red_sem, 1)

            # epilogue: j-reduce psum_big as the last tile's chunks complete
            for mlo, mhi, need in SUBRED:
                vector.wait_ge(pe_big, need)
                vector.tensor_reduce(
                    out=out_sb[:, MV + mlo:MV + mhi],
                    in_=psum_big[:, mlo * HW:mhi * HW].rearrange(
                        "p (m j) -> p m j", j=HW
                    ),
                    axis=mybir.AxisListType.X,
                    op=mybir.AluOpType.add,
                ).then_inc(fin_sem, 1)

        # ---- TensorE: segment-sum accumulation (fp32) ----
        @block.tensor
        def _(tensor):
            tensor.wait_ge(wg_sem, 1)
            for t in range(NTILES):
                slot = t % BUFS
                wt = w_sb[:, t * G:(t + 1) * G]
                prev_pk = None
                for lo, hi, pk in PE_CHUNKS[t]:
                    if pk != prev_pk:
                        piece_done(tensor, t, pk)
                        prev_pk = pk
                    tensor.matmul(
                        out=psum_big[:, lo:hi],
                        lhsT=wt,
                        rhs=x_buf[:, slot * F + FV + lo:slot * F + FV + hi],
                        start=(t == 0 and lo in (0, 512)),
                        stop=(t == NTILES - 1),
                        skip_group_check=True,
                    ).then_inc(pe_big, 1)
                tensor.wait_ge(red_sem, RED_CUM[t])
                tensor.matmul(
                    out=psum_small[:, :],
                    lhsT=wt,
                    rhs=xs_buf[:, slot * MV:(slot + 1) * MV],
                    start=(t == 0),
                    stop=(t == NTILES - 1),
                ).then_inc(pe_tile, 1)

    return nc


def _prepare(x, idx):
    x = np.asarray(x)
    if x.dtype != np.float32:
        x = x.astype(np.float32)
    idx = np.asarray(idx).astype(np.int64)
    counts = np.bincount(idx, minlength=G).astype(np.float64)
    scale = np.where(counts > 0, 1.0 / (counts * HW), 0.0).astype(np.float32)
    aux = np.zeros((P, G + G + NTILES), np.float32)
    aux[:, 0:G] = np.arange(G, dtype=np.float32)[None, :]
    aux[:, G:2 * G] = scale[None, :]
    # per-tile: partition p holds row TILE_START[t] + local(p), or no row
    # (idx -1 -> w row of zeros kills the stale SBUF contents)
    for t in range(NTILES):
        col = np.full(P, -1.0, np.float32)
        row = TILE_START[t]
        for plo, phi in TILE_KINDS[t]:
            nrows = phi - plo
            col[plo:phi] = idx[row:row + nrows].astype(np.float32)
            row += nrows
        aux[:, 2 * G + t] = col
    xr = x.reshape(N, M, HW)
    in_maps = []
    for k in range(CORES):
        shard = np.ascontiguousarray(xr[:, k * ML:(k + 1) * ML, :]).reshape(N, F)
        in_maps.append({"x": shard, "aux": aux})
    return in_maps


def run(x, tensor_list_assignmentindices, trace=False, wait_out=True):
    in_maps = _prepare(x, tensor_list_assignmentindices)
    nc = _build(wait_out=wait_out)
    res = run_bass_kernel_spmd(nc, in_maps, core_ids=list(range(CORES)), trace=trace)
    outs = [np.asarray(r["out"]) for r in res.results]
    out = np.concatenate(outs, axis=1)  # [G, M]
    return out.reshape(G, M, 1, 1).astype(np.float32), res.exec_time_ns


def kernel(**inputs):
    out, _ = run(inputs["x"], inputs["tensor_list_assignmentindices"], trace=False)
    return out
